# revision 29
# baseline (speedup 1.0000x reference)
"""Trainium2 Bass kernel for nn_EdgeDecoder (GNN edge decoder, 2 relations).

Strategy (data-parallel over edges, 8 NeuronCores):
  - Shard the 500k edges of each relation across 8 cores (62500/core).
  - Per (core, relation, sub-shard): host remaps node indices into a compact
    per-shard embedding table (np.unique) so indices fit int16, which enables
    the SWDGE dma_gather instruction (one descriptor per edge instead of one
    instruction per 128 edges). Tables are cast to fp16 on host.
  - On device, per 4096-edge chunk: dma_gather user/item rows (fp16, 256B
    rows), PE-transpose 128x128 blocks to get [dim, edge] layout, then
      hT = relu(W1u^T huT + W1v^T hvT + b1)   (fp16 matmuls, f32 PSUM)
      logits = W2^T hT + b2                   (fp16 matmul,  f32 PSUM)
    and DMA the f32 logits back per chunk. Logits stay sharded; host
    reassembles the full [500000] outputs.
"""
import sys

if "/opt/trn_rl_repo" not in sys.path:
    sys.path.insert(0, "/opt/trn_rl_repo")

import numpy as np

P = 128
D = 128
HID = 256
E = 500000
NCORES = 8
EPC = E // NCORES          # 62500 edges per core per relation
GCH = 4096                 # edges per gather chunk
CCH = 512                  # edges per compute chunk
NREL = 2
NPAIR = 1280               # u-row pairs (2 rows / 512B descriptor) per chunk
NOCT = 128                 # u-row octs (8 rows / 2KB descriptor) per chunk

_PROGRAM_CACHE = {}
LAST_RESULTS = None


def _build_program(nsub, nchunk, tabrows, subl, npair, noct):
    import concourse.bacc as bacc
    import concourse.bass as bass
    import concourse.mybir as mybir
    from concourse.tile import TileContext

    f16, f32, i16 = mybir.dt.float16, mybir.dt.float32, mybir.dt.int16
    subpad = nchunk * GCH
    # real (non-pad) index count per chunk; pads are -1 and the SWDGE ucode
    # stops descriptor generation at the last non-negative index
    counts = [min(GCH, subl - c * GCH) for c in range(nchunk)]
    # full chunks gather noct u-row OCTS (2KB descriptors, 8 adjacent rows),
    # npair PAIRS (512B, 2 rows) and singles; partial chunks are all singles
    nsing_full = GCH - 2 * npair - 8 * noct

    nc = bacc.Bacc("TRN2", target_bir_lowering=False, debug=False,
                   num_swdge_queues=4)

    tabs, idxs_d, outs = {}, {}, {}
    for r in range(NREL):
        for s in range(nsub):
            tabs[("u", r, s)] = nc.dram_tensor(
                f"ut{r}_{s}", [tabrows, D], f16, kind="ExternalInput")
            tabs[("v", r, s)] = nc.dram_tensor(
                f"vt{r}_{s}", [tabrows, D], f16, kind="ExternalInput")
            idxs_d[("u0", r, s)] = nc.dram_tensor(
                f"u0i{r}_{s}", [nchunk, P, max(noct // 16, 1)], i16,
                kind="ExternalInput")
            idxs_d[("u1", r, s)] = nc.dram_tensor(
                f"u1i{r}_{s}", [nchunk, P, max(npair // 16, 1)], i16,
                kind="ExternalInput")
            idxs_d[("u2", r, s)] = nc.dram_tensor(
                f"u2i{r}_{s}", [nchunk, P, GCH // 16], i16, kind="ExternalInput")
            idxs_d[("v", r, s)] = nc.dram_tensor(
                f"vi{r}_{s}", [nchunk, P, GCH // 16], i16, kind="ExternalInput")
        outs[r] = nc.dram_tensor(f"o{r}", [nsub, subpad], f32,
                                 kind="ExternalOutput")
    w1u_d = [nc.dram_tensor(f"w1u{r}", [D, HID], f16, kind="ExternalInput")
             for r in range(NREL)]
    w1v_d = [nc.dram_tensor(f"w1v{r}", [D, HID], f16, kind="ExternalInput")
             for r in range(NREL)]
    w2_d = [nc.dram_tensor(f"w2{r}", [P, 2], f16, kind="ExternalInput")
            for r in range(NREL)]
    b1_d = [nc.dram_tensor(f"b1{r}", [P, 2], f32, kind="ExternalInput")
            for r in range(NREL)]
    b2_d = [nc.dram_tensor(f"b2{r}", [1, 1], f32, kind="ExternalInput")
            for r in range(NREL)]
    id_d = nc.dram_tensor("ident", [P, P], f16, kind="ExternalInput")

    with TileContext(nc) as tc:
        with tc.tile_pool(name="sbw", bufs=1) as sbw, \
             tc.tile_pool(name="sbi", bufs=6) as sbi, \
             tc.tile_pool(name="sbg", bufs=4) as sbg, \
             tc.tile_pool(name="sbt", bufs=3) as sbt, \
             tc.tile_pool(name="sbh", bufs=4) as sbh, \
             tc.tile_pool(name="sblog", bufs=2) as sblog, \
             tc.tile_pool(name="pt", bufs=2, space="PSUM") as pt, \
             tc.tile_pool(name="ph", bufs=3, space="PSUM") as ph, \
             tc.tile_pool(name="pl", bufs=2, space="PSUM") as pl:

            w1u_t, w1v_t, w2_t, b1_t, b2_t = [], [], [], [], []
            for r in range(NREL):
                t = sbw.tile([D, HID], f16, tag=f"w1u{r}")
                nc.sync.dma_start(out=t[:], in_=w1u_d[r].ap()[:])
                w1u_t.append(t)
                t = sbw.tile([D, HID], f16, tag=f"w1v{r}")
                nc.sync.dma_start(out=t[:], in_=w1v_d[r].ap()[:])
                w1v_t.append(t)
                t = sbw.tile([P, 2], f16, tag=f"w2{r}")
                nc.sync.dma_start(out=t[:], in_=w2_d[r].ap()[:])
                w2_t.append(t)
                t = sbw.tile([P, 2], f32, tag=f"b1{r}")
                nc.sync.dma_start(out=t[:], in_=b1_d[r].ap()[:])
                b1_t.append(t)
                t = sbw.tile([1, 1], f32, tag=f"b2{r}")
                nc.sync.dma_start(out=t[:], in_=b2_d[r].ap()[:])
                b2_t.append(t)
            ident = sbw.tile([P, P], f16, tag="ident")
            nc.sync.dma_start(out=ident[:], in_=id_d.ap()[:])

            q = 0
            for r in range(NREL):
                for s in range(nsub):
                    utab = tabs[("u", r, s)]
                    # overlapping f32 view: row stride D fp16, 2 rows (512B =
                    # 128 f32 elems) per read — the SWDGE ucode costs ~8ns per
                    # 128-element unit, so an f32 view makes a 2-row fetch as
                    # cheap as a 1-row one
                    utab_pair = bass.AP(utab.ap().tensor, 0,
                                        [[D, tabrows - 1], [1, 2 * D]]
                                        ).bitcast(mybir.dt.float32)
                    utab_oct = bass.AP(utab.ap().tensor, 0,
                                       [[D, tabrows - 7], [1, 8 * D]]
                                       ).bitcast(mybir.dt.float32)
                    for c in range(nchunk):
                        full = counts[c] == GCH and npair > 0
                        gu = sbg.tile([P, GCH // P, D], f16, tag="gu")
                        ob = 8 * noct // P      # blocks used by octs
                        pb = 2 * npair // P     # blocks used by pairs
                        if full:
                            u0_t = sbi.tile([P, noct // 16], i16, tag="u0")
                            nc.sync.dma_start(out=u0_t[:],
                                              in_=idxs_d[("u0", r, s)].ap()[c])
                            u1_t = sbi.tile([P, npair // 16], i16, tag="u1")
                            nc.sync.dma_start(out=u1_t[:],
                                              in_=idxs_d[("u1", r, s)].ap()[c])
                            u2_t = sbi.tile([P, nsing_full // 16], i16, tag="u2")
                            nc.sync.dma_start(
                                out=u2_t[:],
                                in_=idxs_d[("u2", r, s)].ap()[c][:, :nsing_full // 16])
                            oct_out = gu[:, 0:ob, :].rearrange(
                                "p (a eight) d -> p a (eight d)", eight=8
                            ).bitcast(mybir.dt.float32)
                            nc.gpsimd.dma_gather(
                                oct_out, utab_oct, u0_t[:],
                                noct, noct, 4 * D, elem_step=D // 2,
                                single_packet=False, queue_num=q % 4)
                            pair_out = gu[:, ob:ob + pb, :].rearrange(
                                "p (a two) d -> p a (two d)", two=2
                            ).bitcast(mybir.dt.float32)
                            nc.gpsimd.dma_gather(
                                pair_out, utab_pair, u1_t[:],
                                npair, npair, D, elem_step=D // 2,
                                single_packet=False, queue_num=(q + 1) % 4)
                            nc.gpsimd.dma_gather(
                                gu[:, ob + pb:, :], utab.ap()[:], u2_t[:],
                                nsing_full, nsing_full, D,
                                single_packet=False, queue_num=(q + 2) % 4)
                            qv = (q + 3) % 4
                            q += 4
                        else:
                            u2_t = sbi.tile([P, GCH // 16], i16, tag="u2")
                            nc.sync.dma_start(out=u2_t[:],
                                              in_=idxs_d[("u2", r, s)].ap()[c])
                            nc.gpsimd.dma_gather(
                                gu[:], utab.ap()[:], u2_t[:],
                                GCH, counts[c], D, single_packet=False,
                                queue_num=q % 4)
                            qv = (q + 1) % 4
                            q += 2
                        vi_t = sbi.tile([P, GCH // 16], i16, tag="vi")
                        nc.sync.dma_start(out=vi_t[:],
                                          in_=idxs_d[("v", r, s)].ap()[c])
                        gv = sbg.tile([P, GCH // P, D], f16, tag="gv")
                        nc.gpsimd.dma_gather(
                            gv[:], tabs[("v", r, s)].ap()[:], vi_t[:],
                            GCH, counts[c], D, single_packet=False,
                            queue_num=qv)

                        log_sb = sblog.tile([1, GCH], f32, tag="log")
                        ncc = -(-counts[c] // CCH)
                        for cc in range(ncc):
                            ptu = pt.tile([P, CCH], f16, tag="pt")
                            for j in range(CCH // P):
                                nc.tensor.transpose(
                                    out=ptu[:, j * P:(j + 1) * P],
                                    in_=gu[:, cc * (CCH // P) + j, :],
                                    identity=ident[:])
                            tu = sbt.tile([P, CCH], f16, tag="tu")
                            nc.vector.tensor_copy(out=tu[:], in_=ptu[:])
                            ptv = pt.tile([P, CCH], f16, tag="pt")
                            for j in range(CCH // P):
                                nc.tensor.transpose(
                                    out=ptv[:, j * P:(j + 1) * P],
                                    in_=gv[:, cc * (CCH // P) + j, :],
                                    identity=ident[:])
                            tv = sbt.tile([P, CCH], f16, tag="tv")
                            nc.vector.tensor_copy(out=tv[:], in_=ptv[:])

                            hts = []
                            for hc in range(2):
                                php = ph.tile([P, CCH], f32, tag="ph")
                                nc.tensor.matmul(
                                    out=php[:],
                                    lhsT=w1u_t[r][:, hc * P:(hc + 1) * P],
                                    rhs=tu[:], start=True, stop=False)
                                nc.tensor.matmul(
                                    out=php[:],
                                    lhsT=w1v_t[r][:, hc * P:(hc + 1) * P],
                                    rhs=tv[:], start=False, stop=True)
                                ht = sbh.tile([P, CCH], f16, tag="ht")
                                nc.scalar.activation(
                                    out=ht[:], in_=php[:],
                                    func=mybir.ActivationFunctionType.Relu,
                                    bias=b1_t[r][:, hc:hc + 1])
                                hts.append(ht)
                            plt = pl.tile([1, CCH], f32, tag="pl")
                            nc.tensor.matmul(out=plt[:], lhsT=w2_t[r][:, 0:1],
                                             rhs=hts[0][:], start=True, stop=False)
                            nc.tensor.matmul(out=plt[:], lhsT=w2_t[r][:, 1:2],
                                             rhs=hts[1][:], start=False, stop=True)
                            nc.scalar.activation(
                                out=log_sb[:, cc * CCH:(cc + 1) * CCH],
                                in_=plt[:],
                                func=mybir.ActivationFunctionType.Identity,
                                bias=b2_t[r][:])
                        nc.sync.dma_start(
                            out=outs[r].ap()[s:s + 1,
                                             c * GCH:c * GCH + ncc * CCH],
                            in_=log_sb[:, :ncc * CCH])
    nc.compile()
    return nc


def _wrap16(idx16, nchunk):
    """[subpad] int16 -> [nchunk, 128, GCH//16]: stream pos g of chunk c sits
    at partition g%16 (replicated to all 8 Q7 core groups), column g//16."""
    a = idx16.reshape(nchunk, GCH // 16, 16)
    a = np.swapaxes(a, 1, 2)                       # [nchunk, 16, GCH//16]
    return np.tile(a, (1, 8, 1)).copy()            # [nchunk, 128, GCH//16]


def _wrap16_row(idx16):
    """[n] int16 -> [128, n//16] (16-wrap, replicated to 8 core groups)."""
    a = idx16.reshape(-1, 16).T
    return np.tile(a, (8, 1)).copy()


def _prep(user_embed, item_embed, u_clicks, v_clicks, u_buys, v_buys,
          W1_clicks, b1_clicks, W2_clicks, b2_clicks,
          W1_buys, b1_buys, W2_buys, b2_buys):
    user_embed = np.asarray(user_embed, dtype=np.float32)
    item_embed = np.asarray(item_embed, dtype=np.float32)
    rels = [
        (np.asarray(u_clicks), np.asarray(v_clicks),
         np.asarray(W1_clicks, np.float32), np.asarray(b1_clicks, np.float32),
         np.asarray(W2_clicks, np.float32), np.asarray(b2_clicks, np.float32)),
        (np.asarray(u_buys), np.asarray(v_buys),
         np.asarray(W1_buys, np.float32), np.asarray(b1_buys, np.float32),
         np.asarray(W2_buys, np.float32), np.asarray(b2_buys, np.float32)),
    ]
    user16 = user_embed.astype(np.float16)
    item16 = item_embed.astype(np.float16)

    # pick nsub so every sub-shard's unique index count fits int16
    nsub = 2
    while True:
        subl = EPC // nsub
        ok = True
        for r in range(NREL):
            u_all, v_all = rels[r][0], rels[r][1]
            for k in range(NCORES):
                for s in range(nsub):
                    lo = k * EPC + s * subl
                    hi = lo + subl
                    if len(np.unique(u_all[lo:hi])) > 32700 or \
                       len(np.unique(v_all[lo:hi])) > 32700:
                        ok = False
                        break
                if not ok:
                    break
            if not ok:
                break
        if ok:
            break
        nsub *= 2
        if nsub > 16:
            raise RuntimeError("index space too dense for int16 gather")
    subl = EPC // nsub
    nchunk = -(-subl // GCH)          # chunks per sub-shard
    subpad = nchunk * GCH
    tabrows = 32768

    counts = [min(GCH, subl - c * GCH) for c in range(nchunk)]
    npair = NPAIR
    noct = NOCT
    nsing_full = GCH - 2 * npair - 8 * noct

    def _greedy(d, L, cap, used):
        starts, j, N = [], 0, len(d) + 1
        while j + L <= N and len(starts) < cap:
            if not used[j:j + L].any() and (d[j:j + L - 1] == 1).all():
                starts.append(j)
                used[j:j + L] = True
                j += L
            else:
                j += 1
        return np.asarray(starts, np.int64)

    def _shard(u_sub, v_sub):
        perm = np.argsort(u_sub, kind="stable")
        us, vs = u_sub[perm], v_sub[perm]
        uniq_u, pos = np.unique(us, return_inverse=True)
        uniq_v, vinv = np.unique(vs, return_inverse=True)
        u0 = np.zeros((nchunk, max(noct, 1)), np.int16)
        u1 = np.zeros((nchunk, max(npair, 1)), np.int16)
        u2 = np.full((nchunk, GCH), -1, np.int16)
        v_dev = np.full(subpad, -1, np.int64)
        ood = np.full(subpad, -1, np.int64)
        ob = 8 * noct // P
        pb = 2 * npair // P
        for c in range(nchunk):
            base, cnt = c * GCH, counts[c]
            if cnt == GCH and npair > 0:
                pp = pos[base:base + GCH]
                d = np.diff(pp)
                used = np.zeros(GCH, bool)
                octs = _greedy(d, 8, noct, used)
                pairs = _greedy(d, 2, npair, used)
                if len(octs) < noct or len(pairs) < npair:
                    return None
                sing = np.where(~used)[0]
                u0[c] = pos[base + octs].astype(np.int16)
                u1[c] = pos[base + pairs].astype(np.int16)
                u2[c, :nsing_full] = pos[base + sing].astype(np.int16)
                ks = np.arange(noct)
                js = np.arange(npair)
                ss = np.arange(nsing_full)
                place = []
                for i in range(8):
                    place.append(((8 * (ks // P) + i) * P + ks % P, octs + i))
                gA = (ob + 2 * (js // P)) * P + js % P
                place.append((gA, pairs))
                place.append((gA + P, pairs + 1))
                place.append(((ob + pb + ss // P) * P + ss % P, sing))
                for g, e in place:
                    e = base + e
                    v_dev[base + g] = vinv[e]
                    ood[base + g] = perm[e]
            else:
                e = base + np.arange(cnt)
                u2[c, :cnt] = pos[e].astype(np.int16)
                v_dev[base:base + cnt] = vinv[e]
                ood[base:base + cnt] = perm[e]
        return u0, u1, u2, v_dev, ood, uniq_u, uniq_v

    # verify pairing feasibility on all shards first
    shards = {}
    feasible = True
    for r in range(NREL):
        u_all, v_all = rels[r][0], rels[r][1]
        for k in range(NCORES):
            for s in range(nsub):
                lo = k * EPC + s * subl
                sh = _shard(np.asarray(u_all[lo:lo + subl], np.int64),
                            np.asarray(v_all[lo:lo + subl], np.int64))
                if sh is None:
                    feasible = False
                    break
                shards[(k, r, s)] = sh
            if not feasible:
                break
        if not feasible:
            break
    if not feasible:
        npair = 0
        noct = 0
        nsing_full = GCH
        shards = {}
        for r in range(NREL):
            u_all, v_all = rels[r][0], rels[r][1]
            for k in range(NCORES):
                for s in range(nsub):
                    lo = k * EPC + s * subl
                    shards[(k, r, s)] = _shard(
                        np.asarray(u_all[lo:lo + subl], np.int64),
                        np.asarray(v_all[lo:lo + subl], np.int64))

    in_maps, scat = [], {}
    for k in range(NCORES):
        m = {"ident": np.eye(P, dtype=np.float16)}
        for r in range(NREL):
            u_all, v_all, W1, b1, W2, b2 = rels[r]
            m[f"w1u{r}"] = W1[:D].astype(np.float16)
            m[f"w1v{r}"] = W1[D:].astype(np.float16)
            m[f"w2{r}"] = W2.reshape(2, P).T.astype(np.float16).copy()
            m[f"b1{r}"] = b1.reshape(2, P).T.astype(np.float32).copy()
            m[f"b2{r}"] = b2.reshape(1, 1).astype(np.float32)
            for s in range(nsub):
                u0, u1, u2, v_dev, ood, uniq_u, uniq_v = shards[(k, r, s)]
                comp = np.zeros((tabrows, D), np.float16)
                comp[:len(uniq_u)] = user16[uniq_u]
                m[f"ut{r}_{s}"] = comp
                comp = np.zeros((tabrows, D), np.float16)
                comp[:len(uniq_v)] = item16[uniq_v]
                m[f"vt{r}_{s}"] = comp
                if npair > 0:
                    m[f"u0i{r}_{s}"] = np.stack(
                        [_wrap16_row(u0[c]) for c in range(nchunk)])
                    m[f"u1i{r}_{s}"] = np.stack(
                        [_wrap16_row(u1[c]) for c in range(nchunk)])
                else:
                    m[f"u0i{r}_{s}"] = np.zeros((nchunk, P, 1), np.int16)
                    m[f"u1i{r}_{s}"] = np.zeros((nchunk, P, 1), np.int16)
                m[f"u2i{r}_{s}"] = np.stack(
                    [_wrap16_row(u2[c]) for c in range(nchunk)])
                m[f"vi{r}_{s}"] = _wrap16(
                    np.where(v_dev >= 0, v_dev, -1).astype(np.int16), nchunk)
                scat[(k, r, s)] = ood
        in_maps.append(m)
    return nsub, nchunk, subl, subpad, tabrows, npair, noct, in_maps, scat


def make_in_maps(np_inputs):
    """For external harnesses: per-core input maps for the cached program."""
    return _prep(**np_inputs)[7]


def kernel(**inputs):
    global LAST_RESULTS
    from concourse import bass_utils

    nsub, nchunk, subl, subpad, tabrows, npair, noct, in_maps, scat = _prep(**inputs)

    key = (nsub, nchunk, tabrows, subl, npair, noct)
    if key not in _PROGRAM_CACHE:
        _PROGRAM_CACHE[key] = _build_program(nsub, nchunk, tabrows, subl, npair, noct)
    nc = _PROGRAM_CACHE[key]

    res = bass_utils.run_bass_kernel_spmd(nc, in_maps, core_ids=list(range(NCORES)))
    LAST_RESULTS = res

    outs = []
    for r in range(NREL):
        full = np.empty(E, np.float32)
        for k in range(NCORES):
            o = res.results[k][f"o{r}"]          # [nsub, subpad]
            for s in range(nsub):
                lo = k * EPC + s * subl
                ood = scat[(k, r, s)]
                valid = ood >= 0
                full[lo + ood[valid]] = o[s][valid]
        outs.append(full)
    return outs[0], outs[1]


# revision 32
# speedup vs baseline: 1.0067x; 1.0067x over previous
"""Trainium2 Bass kernel for nn_EdgeDecoder (GNN edge decoder, 2 relations).

Strategy (data-parallel over edges, 8 NeuronCores):
  - Shard the 500k edges of each relation across 8 cores (62500/core).
  - Per (core, relation, sub-shard): host remaps node indices into a compact
    per-shard embedding table (np.unique) so indices fit int16, which enables
    the SWDGE dma_gather instruction (one descriptor per edge instead of one
    instruction per 128 edges). Tables are cast to fp16 on host.
  - On device, per 4096-edge chunk: dma_gather user/item rows (fp16, 256B
    rows), PE-transpose 128x128 blocks to get [dim, edge] layout, then
      hT = relu(W1u^T huT + W1v^T hvT + b1)   (fp16 matmuls, f32 PSUM)
      logits = W2^T hT + b2                   (fp16 matmul,  f32 PSUM)
    and DMA the f32 logits back per chunk. Logits stay sharded; host
    reassembles the full [500000] outputs.
"""
import sys

if "/opt/trn_rl_repo" not in sys.path:
    sys.path.insert(0, "/opt/trn_rl_repo")

import numpy as np

P = 128
D = 128
HID = 256
E = 500000
NCORES = 8
EPC = E // NCORES          # 62500 edges per core per relation
GCH = 4096                 # edges per gather chunk
CCH = 512                  # edges per compute chunk
NREL = 2
NPAIR = 1792               # u-row pairs (2 rows / 512B descriptor) per chunk
NOCT = 0                   # u-row octs: disabled (4-way gather split hurt pipelining)

_PROGRAM_CACHE = {}
LAST_RESULTS = None


def _build_program(nsub, nchunk, tabrows, subl, npair, noct):
    import concourse.bacc as bacc
    import concourse.bass as bass
    import concourse.mybir as mybir
    from concourse.tile import TileContext

    f16, f32, i16 = mybir.dt.float16, mybir.dt.float32, mybir.dt.int16
    subpad = nchunk * GCH
    # real (non-pad) index count per chunk; pads are -1 and the SWDGE ucode
    # stops descriptor generation at the last non-negative index
    counts = [min(GCH, subl - c * GCH) for c in range(nchunk)]
    # full chunks gather noct u-row OCTS (2KB descriptors, 8 adjacent rows),
    # npair PAIRS (512B, 2 rows) and singles; partial chunks are all singles
    nsing_full = GCH - 2 * npair - 8 * noct

    nc = bacc.Bacc("TRN2", target_bir_lowering=False, debug=False,
                   num_swdge_queues=4)

    tabs, idxs_d, outs = {}, {}, {}
    for r in range(NREL):
        for s in range(nsub):
            tabs[("u", r, s)] = nc.dram_tensor(
                f"ut{r}_{s}", [tabrows, D], f16, kind="ExternalInput")
            tabs[("v", r, s)] = nc.dram_tensor(
                f"vt{r}_{s}", [tabrows, D], f16, kind="ExternalInput")
            idxs_d[("u0", r, s)] = nc.dram_tensor(
                f"u0i{r}_{s}", [nchunk, P, max(noct // 16, 1)], i16,
                kind="ExternalInput")
            idxs_d[("u1", r, s)] = nc.dram_tensor(
                f"u1i{r}_{s}", [nchunk, P, max(npair // 16, 1)], i16,
                kind="ExternalInput")
            idxs_d[("u2", r, s)] = nc.dram_tensor(
                f"u2i{r}_{s}", [nchunk, P, GCH // 16], i16, kind="ExternalInput")
            idxs_d[("v", r, s)] = nc.dram_tensor(
                f"vi{r}_{s}", [nchunk, P, GCH // 16], i16, kind="ExternalInput")
        outs[r] = nc.dram_tensor(f"o{r}", [nsub, subpad], f32,
                                 kind="ExternalOutput")
    w1u_d = [nc.dram_tensor(f"w1u{r}", [D, HID], f16, kind="ExternalInput")
             for r in range(NREL)]
    w1v_d = [nc.dram_tensor(f"w1v{r}", [D, HID], f16, kind="ExternalInput")
             for r in range(NREL)]
    w2_d = [nc.dram_tensor(f"w2{r}", [P, 2], f16, kind="ExternalInput")
            for r in range(NREL)]
    b1_d = [nc.dram_tensor(f"b1{r}", [P, 2], f32, kind="ExternalInput")
            for r in range(NREL)]
    b2_d = [nc.dram_tensor(f"b2{r}", [1, 1], f32, kind="ExternalInput")
            for r in range(NREL)]
    id_d = nc.dram_tensor("ident", [P, P], f16, kind="ExternalInput")

    with TileContext(nc) as tc:
        with tc.tile_pool(name="sbw", bufs=1) as sbw, \
             tc.tile_pool(name="sbi", bufs=6) as sbi, \
             tc.tile_pool(name="sbg", bufs=4) as sbg, \
             tc.tile_pool(name="sbt", bufs=3) as sbt, \
             tc.tile_pool(name="sbh", bufs=4) as sbh, \
             tc.tile_pool(name="sblog", bufs=2) as sblog, \
             tc.tile_pool(name="pt", bufs=2, space="PSUM") as pt, \
             tc.tile_pool(name="ph", bufs=3, space="PSUM") as ph, \
             tc.tile_pool(name="pl", bufs=2, space="PSUM") as pl:

            w1u_t, w1v_t, w2_t, b1_t, b2_t = [], [], [], [], []
            for r in range(NREL):
                t = sbw.tile([D, HID], f16, tag=f"w1u{r}")
                nc.sync.dma_start(out=t[:], in_=w1u_d[r].ap()[:])
                w1u_t.append(t)
                t = sbw.tile([D, HID], f16, tag=f"w1v{r}")
                nc.sync.dma_start(out=t[:], in_=w1v_d[r].ap()[:])
                w1v_t.append(t)
                t = sbw.tile([P, 2], f16, tag=f"w2{r}")
                nc.sync.dma_start(out=t[:], in_=w2_d[r].ap()[:])
                w2_t.append(t)
                t = sbw.tile([P, 2], f32, tag=f"b1{r}")
                nc.sync.dma_start(out=t[:], in_=b1_d[r].ap()[:])
                b1_t.append(t)
                t = sbw.tile([1, 1], f32, tag=f"b2{r}")
                nc.sync.dma_start(out=t[:], in_=b2_d[r].ap()[:])
                b2_t.append(t)
            ident = sbw.tile([P, P], f16, tag="ident")
            nc.sync.dma_start(out=ident[:], in_=id_d.ap()[:])

            q = 0
            for r in range(NREL):
                for s in range(nsub):
                    utab = tabs[("u", r, s)]
                    # overlapping f32 view: row stride D fp16, 2 rows (512B =
                    # 128 f32 elems) per read — the SWDGE ucode costs ~8ns per
                    # 128-element unit, so an f32 view makes a 2-row fetch as
                    # cheap as a 1-row one
                    utab_pair = bass.AP(utab.ap().tensor, 0,
                                        [[D, tabrows - 1], [1, 2 * D]]
                                        ).bitcast(mybir.dt.float32)
                    utab_oct = bass.AP(utab.ap().tensor, 0,
                                       [[D, tabrows - 7], [1, 8 * D]]
                                       ).bitcast(mybir.dt.float32)
                    for c in range(nchunk):
                        full = counts[c] == GCH and npair > 0
                        gu = sbg.tile([P, GCH // P, D], f16, tag="gu")
                        ob = 8 * noct // P      # blocks used by octs
                        pb = 2 * npair // P     # blocks used by pairs
                        if full:
                            if noct:
                                u0_t = sbi.tile([P, noct // 16], i16, tag="u0")
                                nc.sync.dma_start(out=u0_t[:],
                                                  in_=idxs_d[("u0", r, s)].ap()[c])
                            u1_t = sbi.tile([P, npair // 16], i16, tag="u1")
                            nc.sync.dma_start(out=u1_t[:],
                                              in_=idxs_d[("u1", r, s)].ap()[c])
                            u2_t = sbi.tile([P, nsing_full // 16], i16, tag="u2")
                            nc.sync.dma_start(
                                out=u2_t[:],
                                in_=idxs_d[("u2", r, s)].ap()[c][:, :nsing_full // 16])
                            if noct:
                                oct_out = gu[:, 0:ob, :].rearrange(
                                    "p (a eight) d -> p a (eight d)", eight=8
                                ).bitcast(mybir.dt.float32)
                                nc.gpsimd.dma_gather(
                                    oct_out, utab_oct, u0_t[:],
                                    noct, noct, 4 * D, elem_step=D // 2,
                                    single_packet=False, queue_num=q % 4)
                            pair_out = gu[:, ob:ob + pb, :].rearrange(
                                "p (a two) d -> p a (two d)", two=2
                            ).bitcast(mybir.dt.float32)
                            nc.gpsimd.dma_gather(
                                pair_out, utab_pair, u1_t[:],
                                npair, npair, D, elem_step=D // 2,
                                single_packet=False, queue_num=(q + 1) % 4)
                            nc.gpsimd.dma_gather(
                                gu[:, ob + pb:, :], utab.ap()[:], u2_t[:],
                                nsing_full, nsing_full, D,
                                single_packet=False, queue_num=(q + 2) % 4)
                            qv = (q + 3) % 4
                            q += 4
                        else:
                            u2_t = sbi.tile([P, GCH // 16], i16, tag="u2")
                            nc.sync.dma_start(out=u2_t[:],
                                              in_=idxs_d[("u2", r, s)].ap()[c])
                            nc.gpsimd.dma_gather(
                                gu[:], utab.ap()[:], u2_t[:],
                                GCH, counts[c], D, single_packet=False,
                                queue_num=q % 4)
                            qv = (q + 1) % 4
                            q += 2
                        vi_t = sbi.tile([P, GCH // 16], i16, tag="vi")
                        nc.sync.dma_start(out=vi_t[:],
                                          in_=idxs_d[("v", r, s)].ap()[c])
                        gv = sbg.tile([P, GCH // P, D], f16, tag="gv")
                        nc.gpsimd.dma_gather(
                            gv[:], tabs[("v", r, s)].ap()[:], vi_t[:],
                            GCH, counts[c], D, single_packet=False,
                            queue_num=qv)

                        log_sb = sblog.tile([1, GCH], f32, tag="log")
                        ncc = -(-counts[c] // CCH)
                        for cc in range(ncc):
                            ptu = pt.tile([P, CCH], f16, tag="pt")
                            for j in range(CCH // P):
                                nc.tensor.transpose(
                                    out=ptu[:, j * P:(j + 1) * P],
                                    in_=gu[:, cc * (CCH // P) + j, :],
                                    identity=ident[:])
                            tu = sbt.tile([P, CCH], f16, tag="tu")
                            nc.vector.tensor_copy(out=tu[:], in_=ptu[:])
                            ptv = pt.tile([P, CCH], f16, tag="pt")
                            for j in range(CCH // P):
                                nc.tensor.transpose(
                                    out=ptv[:, j * P:(j + 1) * P],
                                    in_=gv[:, cc * (CCH // P) + j, :],
                                    identity=ident[:])
                            tv = sbt.tile([P, CCH], f16, tag="tv")
                            nc.vector.tensor_copy(out=tv[:], in_=ptv[:])

                            hts = []
                            for hc in range(2):
                                php = ph.tile([P, CCH], f32, tag="ph")
                                nc.tensor.matmul(
                                    out=php[:],
                                    lhsT=w1u_t[r][:, hc * P:(hc + 1) * P],
                                    rhs=tu[:], start=True, stop=False)
                                nc.tensor.matmul(
                                    out=php[:],
                                    lhsT=w1v_t[r][:, hc * P:(hc + 1) * P],
                                    rhs=tv[:], start=False, stop=True)
                                ht = sbh.tile([P, CCH], f16, tag="ht")
                                nc.scalar.activation(
                                    out=ht[:], in_=php[:],
                                    func=mybir.ActivationFunctionType.Relu,
                                    bias=b1_t[r][:, hc:hc + 1])
                                hts.append(ht)
                            plt = pl.tile([1, CCH], f32, tag="pl")
                            nc.tensor.matmul(out=plt[:], lhsT=w2_t[r][:, 0:1],
                                             rhs=hts[0][:], start=True, stop=False)
                            nc.tensor.matmul(out=plt[:], lhsT=w2_t[r][:, 1:2],
                                             rhs=hts[1][:], start=False, stop=True)
                            nc.scalar.activation(
                                out=log_sb[:, cc * CCH:(cc + 1) * CCH],
                                in_=plt[:],
                                func=mybir.ActivationFunctionType.Identity,
                                bias=b2_t[r][:])
                        nc.sync.dma_start(
                            out=outs[r].ap()[s:s + 1,
                                             c * GCH:c * GCH + ncc * CCH],
                            in_=log_sb[:, :ncc * CCH])
    nc.compile()
    return nc


def _wrap16(idx16, nchunk):
    """[subpad] int16 -> [nchunk, 128, GCH//16]: stream pos g of chunk c sits
    at partition g%16 (replicated to all 8 Q7 core groups), column g//16."""
    a = idx16.reshape(nchunk, GCH // 16, 16)
    a = np.swapaxes(a, 1, 2)                       # [nchunk, 16, GCH//16]
    return np.tile(a, (1, 8, 1)).copy()            # [nchunk, 128, GCH//16]


def _wrap16_row(idx16):
    """[n] int16 -> [128, n//16] (16-wrap, replicated to 8 core groups)."""
    a = idx16.reshape(-1, 16).T
    return np.tile(a, (8, 1)).copy()


def _prep(user_embed, item_embed, u_clicks, v_clicks, u_buys, v_buys,
          W1_clicks, b1_clicks, W2_clicks, b2_clicks,
          W1_buys, b1_buys, W2_buys, b2_buys):
    user_embed = np.asarray(user_embed, dtype=np.float32)
    item_embed = np.asarray(item_embed, dtype=np.float32)
    rels = [
        (np.asarray(u_clicks), np.asarray(v_clicks),
         np.asarray(W1_clicks, np.float32), np.asarray(b1_clicks, np.float32),
         np.asarray(W2_clicks, np.float32), np.asarray(b2_clicks, np.float32)),
        (np.asarray(u_buys), np.asarray(v_buys),
         np.asarray(W1_buys, np.float32), np.asarray(b1_buys, np.float32),
         np.asarray(W2_buys, np.float32), np.asarray(b2_buys, np.float32)),
    ]
    user16 = user_embed.astype(np.float16)
    item16 = item_embed.astype(np.float16)

    # pick nsub so every sub-shard's unique index count fits int16
    nsub = 2
    while True:
        subl = EPC // nsub
        ok = True
        for r in range(NREL):
            u_all, v_all = rels[r][0], rels[r][1]
            for k in range(NCORES):
                for s in range(nsub):
                    lo = k * EPC + s * subl
                    hi = lo + subl
                    if len(np.unique(u_all[lo:hi])) > 32700 or \
                       len(np.unique(v_all[lo:hi])) > 32700:
                        ok = False
                        break
                if not ok:
                    break
            if not ok:
                break
        if ok:
            break
        nsub *= 2
        if nsub > 16:
            raise RuntimeError("index space too dense for int16 gather")
    subl = EPC // nsub
    nchunk = -(-subl // GCH)          # chunks per sub-shard
    subpad = nchunk * GCH
    tabrows = 32768

    counts = [min(GCH, subl - c * GCH) for c in range(nchunk)]
    npair = NPAIR
    noct = NOCT
    nsing_full = GCH - 2 * npair - 8 * noct

    def _greedy(d, L, cap, used):
        starts, j, N = [], 0, len(d) + 1
        while j + L <= N and len(starts) < cap:
            if not used[j:j + L].any() and (d[j:j + L - 1] == 1).all():
                starts.append(j)
                used[j:j + L] = True
                j += L
            else:
                j += 1
        return np.asarray(starts, np.int64)

    def _shard(u_sub, v_sub):
        perm = np.argsort(u_sub, kind="stable")
        us, vs = u_sub[perm], v_sub[perm]
        uniq_u, pos = np.unique(us, return_inverse=True)
        uniq_v, vinv = np.unique(vs, return_inverse=True)
        u0 = np.zeros((nchunk, max(noct, 1)), np.int16)
        u1 = np.zeros((nchunk, max(npair, 1)), np.int16)
        u2 = np.full((nchunk, GCH), -1, np.int16)
        v_dev = np.full(subpad, -1, np.int64)
        ood = np.full(subpad, -1, np.int64)
        ob = 8 * noct // P
        pb = 2 * npair // P
        for c in range(nchunk):
            base, cnt = c * GCH, counts[c]
            if cnt == GCH and npair > 0:
                pp = pos[base:base + GCH]
                d = np.diff(pp)
                used = np.zeros(GCH, bool)
                octs = _greedy(d, 8, noct, used)
                pairs = _greedy(d, 2, npair, used)
                if len(octs) < noct or len(pairs) < npair:
                    return None
                sing = np.where(~used)[0]
                if noct:
                    u0[c] = pos[base + octs].astype(np.int16)
                u1[c] = pos[base + pairs].astype(np.int16)
                u2[c, :nsing_full] = pos[base + sing].astype(np.int16)
                ks = np.arange(noct)
                js = np.arange(npair)
                ss = np.arange(nsing_full)
                place = []
                for i in range(8):
                    place.append(((8 * (ks // P) + i) * P + ks % P, octs + i))
                gA = (ob + 2 * (js // P)) * P + js % P
                place.append((gA, pairs))
                place.append((gA + P, pairs + 1))
                place.append(((ob + pb + ss // P) * P + ss % P, sing))
                for g, e in place:
                    e = base + e
                    v_dev[base + g] = vinv[e]
                    ood[base + g] = perm[e]
            else:
                e = base + np.arange(cnt)
                u2[c, :cnt] = pos[e].astype(np.int16)
                v_dev[base:base + cnt] = vinv[e]
                ood[base:base + cnt] = perm[e]
        return u0, u1, u2, v_dev, ood, uniq_u, uniq_v

    # verify pairing feasibility on all shards first
    shards = {}
    feasible = True
    for r in range(NREL):
        u_all, v_all = rels[r][0], rels[r][1]
        for k in range(NCORES):
            for s in range(nsub):
                lo = k * EPC + s * subl
                sh = _shard(np.asarray(u_all[lo:lo + subl], np.int64),
                            np.asarray(v_all[lo:lo + subl], np.int64))
                if sh is None:
                    feasible = False
                    break
                shards[(k, r, s)] = sh
            if not feasible:
                break
        if not feasible:
            break
    if not feasible:
        npair = 0
        noct = 0
        nsing_full = GCH
        shards = {}
        for r in range(NREL):
            u_all, v_all = rels[r][0], rels[r][1]
            for k in range(NCORES):
                for s in range(nsub):
                    lo = k * EPC + s * subl
                    shards[(k, r, s)] = _shard(
                        np.asarray(u_all[lo:lo + subl], np.int64),
                        np.asarray(v_all[lo:lo + subl], np.int64))

    in_maps, scat = [], {}
    for k in range(NCORES):
        m = {"ident": np.eye(P, dtype=np.float16)}
        for r in range(NREL):
            u_all, v_all, W1, b1, W2, b2 = rels[r]
            m[f"w1u{r}"] = W1[:D].astype(np.float16)
            m[f"w1v{r}"] = W1[D:].astype(np.float16)
            m[f"w2{r}"] = W2.reshape(2, P).T.astype(np.float16).copy()
            m[f"b1{r}"] = b1.reshape(2, P).T.astype(np.float32).copy()
            m[f"b2{r}"] = b2.reshape(1, 1).astype(np.float32)
            for s in range(nsub):
                u0, u1, u2, v_dev, ood, uniq_u, uniq_v = shards[(k, r, s)]
                comp = np.zeros((tabrows, D), np.float16)
                comp[:len(uniq_u)] = user16[uniq_u]
                m[f"ut{r}_{s}"] = comp
                comp = np.zeros((tabrows, D), np.float16)
                comp[:len(uniq_v)] = item16[uniq_v]
                m[f"vt{r}_{s}"] = comp
                if npair > 0:
                    if noct > 0:
                        m[f"u0i{r}_{s}"] = np.stack(
                            [_wrap16_row(u0[c]) for c in range(nchunk)])
                    else:
                        m[f"u0i{r}_{s}"] = np.zeros((nchunk, P, 1), np.int16)
                    m[f"u1i{r}_{s}"] = np.stack(
                        [_wrap16_row(u1[c]) for c in range(nchunk)])
                else:
                    m[f"u0i{r}_{s}"] = np.zeros((nchunk, P, 1), np.int16)
                    m[f"u1i{r}_{s}"] = np.zeros((nchunk, P, 1), np.int16)
                m[f"u2i{r}_{s}"] = np.stack(
                    [_wrap16_row(u2[c]) for c in range(nchunk)])
                m[f"vi{r}_{s}"] = _wrap16(
                    np.where(v_dev >= 0, v_dev, -1).astype(np.int16), nchunk)
                scat[(k, r, s)] = ood
        in_maps.append(m)
    return nsub, nchunk, subl, subpad, tabrows, npair, noct, in_maps, scat


def make_in_maps(np_inputs):
    """For external harnesses: per-core input maps for the cached program."""
    return _prep(**np_inputs)[7]


def kernel(**inputs):
    global LAST_RESULTS
    from concourse import bass_utils

    nsub, nchunk, subl, subpad, tabrows, npair, noct, in_maps, scat = _prep(**inputs)

    key = (nsub, nchunk, tabrows, subl, npair, noct)
    if key not in _PROGRAM_CACHE:
        _PROGRAM_CACHE[key] = _build_program(nsub, nchunk, tabrows, subl, npair, noct)
    nc = _PROGRAM_CACHE[key]

    res = bass_utils.run_bass_kernel_spmd(nc, in_maps, core_ids=list(range(NCORES)))
    LAST_RESULTS = res

    outs = []
    for r in range(NREL):
        full = np.empty(E, np.float32)
        for k in range(NCORES):
            o = res.results[k][f"o{r}"]          # [nsub, subpad]
            for s in range(nsub):
                lo = k * EPC + s * subl
                ood = scat[(k, r, s)]
                valid = ood >= 0
                full[lo + ood[valid]] = o[s][valid]
        outs.append(full)
    return outs[0], outs[1]


# revision 33
# speedup vs baseline: 1.2750x; 1.2664x over previous
"""Trainium2 Bass kernel for nn_EdgeDecoder (GNN edge decoder, 2 relations).

Strategy (data-parallel over edges, 8 NeuronCores):
  - Shard the 500k edges of each relation across 8 cores (62500/core).
  - Per (core, relation, sub-shard): host remaps node indices into a compact
    per-shard embedding table (np.unique) so indices fit int16, which enables
    the SWDGE dma_gather instruction (one descriptor per edge instead of one
    instruction per 128 edges). Tables are cast to fp16 on host.
  - On device, per 4096-edge chunk: dma_gather user/item rows (fp16, 256B
    rows), PE-transpose 128x128 blocks to get [dim, edge] layout, then
      hT = relu(W1u^T huT + W1v^T hvT + b1)   (fp16 matmuls, f32 PSUM)
      logits = W2^T hT + b2                   (fp16 matmul,  f32 PSUM)
    and DMA the f32 logits back per chunk. Logits stay sharded; host
    reassembles the full [500000] outputs.
"""
import sys

if "/opt/trn_rl_repo" not in sys.path:
    sys.path.insert(0, "/opt/trn_rl_repo")

import numpy as np

P = 128
D = 128
HID = 256
E = 500000
NCORES = 8
EPC = E // NCORES          # 62500 edges per core per relation
GCH = 4096                 # edges per gather chunk
CCH = 512                  # edges per compute chunk
NREL = 2
NPAIR = 1792               # u-row pairs (2 rows / 512B descriptor) per chunk
NOCT = 0                   # u-row octs: disabled (4-way gather split hurt pipelining)

_PROGRAM_CACHE = {}
LAST_RESULTS = None


def _build_program(nsub, nchunk, tabrows, subl, npair, noct):
    import concourse.bacc as bacc
    import concourse.bass as bass
    import concourse.mybir as mybir
    from concourse.tile import TileContext

    f16, f32, i16 = mybir.dt.float16, mybir.dt.float32, mybir.dt.int16
    subpad = nchunk * GCH
    # real (non-pad) index count per chunk; pads are -1 and the SWDGE ucode
    # stops descriptor generation at the last non-negative index
    counts = [min(GCH, subl - c * GCH) for c in range(nchunk)]
    # full chunks gather noct u-row OCTS (2KB descriptors, 8 adjacent rows),
    # npair PAIRS (512B, 2 rows) and singles; partial chunks are all singles
    nsing_full = GCH - 2 * npair - 8 * noct

    nc = bacc.Bacc("TRN2", target_bir_lowering=False, debug=False,
                   num_swdge_queues=4)

    tabs, idxs_d, outs = {}, {}, {}
    for r in range(NREL):
        for s in range(nsub):
            tabs[("u", r, s)] = nc.dram_tensor(
                f"ut{r}_{s}", [tabrows, D], f16, kind="ExternalInput")
            tabs[("v", r, s)] = nc.dram_tensor(
                f"vt{r}_{s}", [tabrows, D], f16, kind="ExternalInput")
            idxs_d[("u0", r, s)] = nc.dram_tensor(
                f"u0i{r}_{s}", [nchunk, P, max(noct // 16, 1)], i16,
                kind="ExternalInput")
            idxs_d[("u1", r, s)] = nc.dram_tensor(
                f"u1i{r}_{s}", [nchunk, P, max(npair // 16, 1)], i16,
                kind="ExternalInput")
            idxs_d[("u2", r, s)] = nc.dram_tensor(
                f"u2i{r}_{s}", [nchunk, P, GCH // 16], i16, kind="ExternalInput")
            idxs_d[("v", r, s)] = nc.dram_tensor(
                f"vi{r}_{s}", [nchunk, P, GCH // 16], i16, kind="ExternalInput")
        outs[r] = nc.dram_tensor(f"o{r}", [nsub, subpad], f32,
                                 kind="ExternalOutput")
    w1u_d = [nc.dram_tensor(f"w1u{r}", [D, HID], f16, kind="ExternalInput")
             for r in range(NREL)]
    w1v_d = [nc.dram_tensor(f"w1v{r}", [D, HID], f16, kind="ExternalInput")
             for r in range(NREL)]
    w2_d = [nc.dram_tensor(f"w2{r}", [P, 2], f16, kind="ExternalInput")
            for r in range(NREL)]
    b1_d = [nc.dram_tensor(f"b1{r}", [P, 2], f32, kind="ExternalInput")
            for r in range(NREL)]
    b2_d = [nc.dram_tensor(f"b2{r}", [1, 1], f32, kind="ExternalInput")
            for r in range(NREL)]
    id_d = nc.dram_tensor("ident", [P, P], f16, kind="ExternalInput")

    with TileContext(nc) as tc:
        with tc.tile_pool(name="sbw", bufs=1) as sbw, \
             tc.tile_pool(name="sbi", bufs=6) as sbi, \
             tc.tile_pool(name="sbg", bufs=4) as sbg, \
             tc.tile_pool(name="sbt", bufs=3) as sbt, \
             tc.tile_pool(name="sbh", bufs=4) as sbh, \
             tc.tile_pool(name="sblog", bufs=2) as sblog, \
             tc.tile_pool(name="pt", bufs=2, space="PSUM") as pt, \
             tc.tile_pool(name="ph", bufs=3, space="PSUM") as ph, \
             tc.tile_pool(name="pl", bufs=2, space="PSUM") as pl:

            w1u_t, w1v_t, w2_t, b1_t, b2_t = [], [], [], [], []
            for r in range(NREL):
                t = sbw.tile([D, HID], f16, tag=f"w1u{r}")
                nc.sync.dma_start(out=t[:], in_=w1u_d[r].ap()[:])
                w1u_t.append(t)
                t = sbw.tile([D, HID], f16, tag=f"w1v{r}")
                nc.sync.dma_start(out=t[:], in_=w1v_d[r].ap()[:])
                w1v_t.append(t)
                t = sbw.tile([P, 2], f16, tag=f"w2{r}")
                nc.sync.dma_start(out=t[:], in_=w2_d[r].ap()[:])
                w2_t.append(t)
                t = sbw.tile([P, 2], f32, tag=f"b1{r}")
                nc.sync.dma_start(out=t[:], in_=b1_d[r].ap()[:])
                b1_t.append(t)
                t = sbw.tile([1, 1], f32, tag=f"b2{r}")
                nc.sync.dma_start(out=t[:], in_=b2_d[r].ap()[:])
                b2_t.append(t)
            ident = sbw.tile([P, P], f16, tag="ident")
            nc.sync.dma_start(out=ident[:], in_=id_d.ap()[:])

            q = 0
            for r in range(NREL):
                for s in range(nsub):
                    utab = tabs[("u", r, s)]
                    # overlapping f32 view: row stride D fp16, 2 rows (512B =
                    # 128 f32 elems) per read — the SWDGE ucode costs ~8ns per
                    # 128-element unit, so an f32 view makes a 2-row fetch as
                    # cheap as a 1-row one
                    utab_pair = bass.AP(utab.ap().tensor, 0,
                                        [[D, tabrows - 1], [1, 2 * D]]
                                        ).bitcast(mybir.dt.float32)
                    utab_oct = bass.AP(utab.ap().tensor, 0,
                                       [[D, tabrows - 7], [1, 8 * D]]
                                       ).bitcast(mybir.dt.float32)
                    for c in range(nchunk):
                        full = counts[c] == GCH and npair > 0
                        gu = sbg.tile([P, GCH // P, D], f16, tag="gu")
                        ob = 8 * noct // P      # blocks used by octs
                        pb = 2 * npair // P     # blocks used by pairs
                        if full:
                            if noct:
                                u0_t = sbi.tile([P, noct // 16], i16, tag="u0")
                                nc.sync.dma_start(out=u0_t[:],
                                                  in_=idxs_d[("u0", r, s)].ap()[c])
                            u1_t = sbi.tile([P, npair // 16], i16, tag="u1")
                            nc.sync.dma_start(out=u1_t[:],
                                              in_=idxs_d[("u1", r, s)].ap()[c])
                            u2_t = sbi.tile([P, nsing_full // 16], i16, tag="u2")
                            nc.sync.dma_start(
                                out=u2_t[:],
                                in_=idxs_d[("u2", r, s)].ap()[c][:, :nsing_full // 16])
                            if noct:
                                oct_out = gu[:, 0:ob, :].rearrange(
                                    "p (a eight) d -> p a (eight d)", eight=8
                                ).bitcast(mybir.dt.float32)
                                nc.gpsimd.dma_gather(
                                    oct_out, utab_oct, u0_t[:],
                                    noct, noct, 4 * D, elem_step=D // 2,
                                    single_packet=False, queue_num=q % 4)
                            pair_out = gu[:, ob:ob + pb, :].rearrange(
                                "p (a two) d -> p a (two d)", two=2
                            ).bitcast(mybir.dt.float32)
                            no = 1 if noct else 0
                            nc.gpsimd.dma_gather(
                                pair_out, utab_pair, u1_t[:],
                                npair, npair, D, elem_step=D // 2,
                                single_packet=False, queue_num=(q + no) % 4)
                            nc.gpsimd.dma_gather(
                                gu[:, ob + pb:, :], utab.ap()[:], u2_t[:],
                                nsing_full, nsing_full, D,
                                single_packet=False, queue_num=(q + no + 1) % 4)
                            qv = (q + no + 2) % 4
                            q += no + 3
                        else:
                            u2_t = sbi.tile([P, GCH // 16], i16, tag="u2")
                            nc.sync.dma_start(out=u2_t[:],
                                              in_=idxs_d[("u2", r, s)].ap()[c])
                            nc.gpsimd.dma_gather(
                                gu[:], utab.ap()[:], u2_t[:],
                                GCH, counts[c], D, single_packet=False,
                                queue_num=q % 4)
                            qv = (q + 1) % 4
                            q += 2
                        vi_t = sbi.tile([P, GCH // 16], i16, tag="vi")
                        nc.sync.dma_start(out=vi_t[:],
                                          in_=idxs_d[("v", r, s)].ap()[c])
                        gv = sbg.tile([P, GCH // P, D], f16, tag="gv")
                        nc.gpsimd.dma_gather(
                            gv[:], tabs[("v", r, s)].ap()[:], vi_t[:],
                            GCH, counts[c], D, single_packet=False,
                            queue_num=qv)

                        log_sb = sblog.tile([1, GCH], f32, tag="log")
                        ncc = -(-counts[c] // CCH)
                        for cc in range(ncc):
                            ptu = pt.tile([P, CCH], f16, tag="pt")
                            for j in range(CCH // P):
                                nc.tensor.transpose(
                                    out=ptu[:, j * P:(j + 1) * P],
                                    in_=gu[:, cc * (CCH // P) + j, :],
                                    identity=ident[:])
                            tu = sbt.tile([P, CCH], f16, tag="tu")
                            nc.vector.tensor_copy(out=tu[:], in_=ptu[:])
                            ptv = pt.tile([P, CCH], f16, tag="pt")
                            for j in range(CCH // P):
                                nc.tensor.transpose(
                                    out=ptv[:, j * P:(j + 1) * P],
                                    in_=gv[:, cc * (CCH // P) + j, :],
                                    identity=ident[:])
                            tv = sbt.tile([P, CCH], f16, tag="tv")
                            nc.vector.tensor_copy(out=tv[:], in_=ptv[:])

                            hts = []
                            for hc in range(2):
                                php = ph.tile([P, CCH], f32, tag="ph")
                                nc.tensor.matmul(
                                    out=php[:],
                                    lhsT=w1u_t[r][:, hc * P:(hc + 1) * P],
                                    rhs=tu[:], start=True, stop=False)
                                nc.tensor.matmul(
                                    out=php[:],
                                    lhsT=w1v_t[r][:, hc * P:(hc + 1) * P],
                                    rhs=tv[:], start=False, stop=True)
                                ht = sbh.tile([P, CCH], f16, tag="ht")
                                nc.scalar.activation(
                                    out=ht[:], in_=php[:],
                                    func=mybir.ActivationFunctionType.Relu,
                                    bias=b1_t[r][:, hc:hc + 1])
                                hts.append(ht)
                            plt = pl.tile([1, CCH], f32, tag="pl")
                            nc.tensor.matmul(out=plt[:], lhsT=w2_t[r][:, 0:1],
                                             rhs=hts[0][:], start=True, stop=False)
                            nc.tensor.matmul(out=plt[:], lhsT=w2_t[r][:, 1:2],
                                             rhs=hts[1][:], start=False, stop=True)
                            nc.scalar.activation(
                                out=log_sb[:, cc * CCH:(cc + 1) * CCH],
                                in_=plt[:],
                                func=mybir.ActivationFunctionType.Identity,
                                bias=b2_t[r][:])
                        nc.sync.dma_start(
                            out=outs[r].ap()[s:s + 1,
                                             c * GCH:c * GCH + ncc * CCH],
                            in_=log_sb[:, :ncc * CCH])
    nc.compile()
    return nc


def _wrap16(idx16, nchunk):
    """[subpad] int16 -> [nchunk, 128, GCH//16]: stream pos g of chunk c sits
    at partition g%16 (replicated to all 8 Q7 core groups), column g//16."""
    a = idx16.reshape(nchunk, GCH // 16, 16)
    a = np.swapaxes(a, 1, 2)                       # [nchunk, 16, GCH//16]
    return np.tile(a, (1, 8, 1)).copy()            # [nchunk, 128, GCH//16]


def _wrap16_row(idx16):
    """[n] int16 -> [128, n//16] (16-wrap, replicated to 8 core groups)."""
    a = idx16.reshape(-1, 16).T
    return np.tile(a, (8, 1)).copy()


def _prep(user_embed, item_embed, u_clicks, v_clicks, u_buys, v_buys,
          W1_clicks, b1_clicks, W2_clicks, b2_clicks,
          W1_buys, b1_buys, W2_buys, b2_buys):
    user_embed = np.asarray(user_embed, dtype=np.float32)
    item_embed = np.asarray(item_embed, dtype=np.float32)
    rels = [
        (np.asarray(u_clicks), np.asarray(v_clicks),
         np.asarray(W1_clicks, np.float32), np.asarray(b1_clicks, np.float32),
         np.asarray(W2_clicks, np.float32), np.asarray(b2_clicks, np.float32)),
        (np.asarray(u_buys), np.asarray(v_buys),
         np.asarray(W1_buys, np.float32), np.asarray(b1_buys, np.float32),
         np.asarray(W2_buys, np.float32), np.asarray(b2_buys, np.float32)),
    ]
    user16 = user_embed.astype(np.float16)
    item16 = item_embed.astype(np.float16)

    # pick nsub so every sub-shard's unique index count fits int16
    nsub = 2
    while True:
        subl = EPC // nsub
        ok = True
        for r in range(NREL):
            u_all, v_all = rels[r][0], rels[r][1]
            for k in range(NCORES):
                for s in range(nsub):
                    lo = k * EPC + s * subl
                    hi = lo + subl
                    if len(np.unique(u_all[lo:hi])) > 32700 or \
                       len(np.unique(v_all[lo:hi])) > 32700:
                        ok = False
                        break
                if not ok:
                    break
            if not ok:
                break
        if ok:
            break
        nsub *= 2
        if nsub > 16:
            raise RuntimeError("index space too dense for int16 gather")
    subl = EPC // nsub
    nchunk = -(-subl // GCH)          # chunks per sub-shard
    subpad = nchunk * GCH
    tabrows = 32768

    counts = [min(GCH, subl - c * GCH) for c in range(nchunk)]
    npair = NPAIR
    noct = NOCT
    nsing_full = GCH - 2 * npair - 8 * noct

    def _greedy(d, L, cap, used):
        starts, j, N = [], 0, len(d) + 1
        while j + L <= N and len(starts) < cap:
            if not used[j:j + L].any() and (d[j:j + L - 1] == 1).all():
                starts.append(j)
                used[j:j + L] = True
                j += L
            else:
                j += 1
        return np.asarray(starts, np.int64)

    def _shard(u_sub, v_sub):
        perm = np.argsort(u_sub, kind="stable")
        us, vs = u_sub[perm], v_sub[perm]
        uniq_u, pos = np.unique(us, return_inverse=True)
        uniq_v, vinv = np.unique(vs, return_inverse=True)
        u0 = np.zeros((nchunk, max(noct, 1)), np.int16)
        u1 = np.zeros((nchunk, max(npair, 1)), np.int16)
        u2 = np.full((nchunk, GCH), -1, np.int16)
        v_dev = np.full(subpad, -1, np.int64)
        ood = np.full(subpad, -1, np.int64)
        ob = 8 * noct // P
        pb = 2 * npair // P
        for c in range(nchunk):
            base, cnt = c * GCH, counts[c]
            if cnt == GCH and npair > 0:
                pp = pos[base:base + GCH]
                d = np.diff(pp)
                used = np.zeros(GCH, bool)
                octs = _greedy(d, 8, noct, used)
                pairs = _greedy(d, 2, npair, used)
                if len(octs) < noct or len(pairs) < npair:
                    return None
                sing = np.where(~used)[0]
                if noct:
                    u0[c] = pos[base + octs].astype(np.int16)
                u1[c] = pos[base + pairs].astype(np.int16)
                u2[c, :nsing_full] = pos[base + sing].astype(np.int16)
                ks = np.arange(noct)
                js = np.arange(npair)
                ss = np.arange(nsing_full)
                place = []
                for i in range(8):
                    place.append(((8 * (ks // P) + i) * P + ks % P, octs + i))
                gA = (ob + 2 * (js // P)) * P + js % P
                place.append((gA, pairs))
                place.append((gA + P, pairs + 1))
                place.append(((ob + pb + ss // P) * P + ss % P, sing))
                for g, e in place:
                    e = base + e
                    v_dev[base + g] = vinv[e]
                    ood[base + g] = perm[e]
            else:
                e = base + np.arange(cnt)
                u2[c, :cnt] = pos[e].astype(np.int16)
                v_dev[base:base + cnt] = vinv[e]
                ood[base:base + cnt] = perm[e]
        return u0, u1, u2, v_dev, ood, uniq_u, uniq_v

    # verify pairing feasibility on all shards first
    shards = {}
    feasible = True
    for r in range(NREL):
        u_all, v_all = rels[r][0], rels[r][1]
        for k in range(NCORES):
            for s in range(nsub):
                lo = k * EPC + s * subl
                sh = _shard(np.asarray(u_all[lo:lo + subl], np.int64),
                            np.asarray(v_all[lo:lo + subl], np.int64))
                if sh is None:
                    feasible = False
                    break
                shards[(k, r, s)] = sh
            if not feasible:
                break
        if not feasible:
            break
    if not feasible:
        npair = 0
        noct = 0
        nsing_full = GCH
        shards = {}
        for r in range(NREL):
            u_all, v_all = rels[r][0], rels[r][1]
            for k in range(NCORES):
                for s in range(nsub):
                    lo = k * EPC + s * subl
                    shards[(k, r, s)] = _shard(
                        np.asarray(u_all[lo:lo + subl], np.int64),
                        np.asarray(v_all[lo:lo + subl], np.int64))

    in_maps, scat = [], {}
    for k in range(NCORES):
        m = {"ident": np.eye(P, dtype=np.float16)}
        for r in range(NREL):
            u_all, v_all, W1, b1, W2, b2 = rels[r]
            m[f"w1u{r}"] = W1[:D].astype(np.float16)
            m[f"w1v{r}"] = W1[D:].astype(np.float16)
            m[f"w2{r}"] = W2.reshape(2, P).T.astype(np.float16).copy()
            m[f"b1{r}"] = b1.reshape(2, P).T.astype(np.float32).copy()
            m[f"b2{r}"] = b2.reshape(1, 1).astype(np.float32)
            for s in range(nsub):
                u0, u1, u2, v_dev, ood, uniq_u, uniq_v = shards[(k, r, s)]
                comp = np.zeros((tabrows, D), np.float16)
                comp[:len(uniq_u)] = user16[uniq_u]
                m[f"ut{r}_{s}"] = comp
                comp = np.zeros((tabrows, D), np.float16)
                comp[:len(uniq_v)] = item16[uniq_v]
                m[f"vt{r}_{s}"] = comp
                if npair > 0:
                    if noct > 0:
                        m[f"u0i{r}_{s}"] = np.stack(
                            [_wrap16_row(u0[c]) for c in range(nchunk)])
                    else:
                        m[f"u0i{r}_{s}"] = np.zeros((nchunk, P, 1), np.int16)
                    m[f"u1i{r}_{s}"] = np.stack(
                        [_wrap16_row(u1[c]) for c in range(nchunk)])
                else:
                    m[f"u0i{r}_{s}"] = np.zeros((nchunk, P, 1), np.int16)
                    m[f"u1i{r}_{s}"] = np.zeros((nchunk, P, 1), np.int16)
                m[f"u2i{r}_{s}"] = np.stack(
                    [_wrap16_row(u2[c]) for c in range(nchunk)])
                m[f"vi{r}_{s}"] = _wrap16(
                    np.where(v_dev >= 0, v_dev, -1).astype(np.int16), nchunk)
                scat[(k, r, s)] = ood
        in_maps.append(m)
    return nsub, nchunk, subl, subpad, tabrows, npair, noct, in_maps, scat


def make_in_maps(np_inputs):
    """For external harnesses: per-core input maps for the cached program."""
    return _prep(**np_inputs)[7]


def kernel(**inputs):
    global LAST_RESULTS
    from concourse import bass_utils

    nsub, nchunk, subl, subpad, tabrows, npair, noct, in_maps, scat = _prep(**inputs)

    key = (nsub, nchunk, tabrows, subl, npair, noct)
    if key not in _PROGRAM_CACHE:
        _PROGRAM_CACHE[key] = _build_program(nsub, nchunk, tabrows, subl, npair, noct)
    nc = _PROGRAM_CACHE[key]

    res = bass_utils.run_bass_kernel_spmd(nc, in_maps, core_ids=list(range(NCORES)))
    LAST_RESULTS = res

    outs = []
    for r in range(NREL):
        full = np.empty(E, np.float32)
        for k in range(NCORES):
            o = res.results[k][f"o{r}"]          # [nsub, subpad]
            for s in range(nsub):
                lo = k * EPC + s * subl
                ood = scat[(k, r, s)]
                valid = ood >= 0
                full[lo + ood[valid]] = o[s][valid]
        outs.append(full)
    return outs[0], outs[1]


# revision 34
# speedup vs baseline: 1.5092x; 1.1837x over previous
"""Trainium2 Bass kernel for nn_EdgeDecoder (GNN edge decoder, 2 relations).

Strategy (data-parallel over edges, 8 NeuronCores):
  - Shard the 500k edges of each relation across 8 cores (62500/core).
  - Per (core, relation, sub-shard): host remaps node indices into a compact
    per-shard embedding table (np.unique) so indices fit int16, which enables
    the SWDGE dma_gather instruction (one descriptor per edge instead of one
    instruction per 128 edges). Tables are cast to fp16 on host.
  - On device, per 4096-edge chunk: dma_gather user/item rows (fp16, 256B
    rows), PE-transpose 128x128 blocks to get [dim, edge] layout, then
      hT = relu(W1u^T huT + W1v^T hvT + b1)   (fp16 matmuls, f32 PSUM)
      logits = W2^T hT + b2                   (fp16 matmul,  f32 PSUM)
    and DMA the f32 logits back per chunk. Logits stay sharded; host
    reassembles the full [500000] outputs.
"""
import sys

if "/opt/trn_rl_repo" not in sys.path:
    sys.path.insert(0, "/opt/trn_rl_repo")

import numpy as np

P = 128
D = 128
HID = 256
E = 500000
NCORES = 8
EPC = E // NCORES          # 62500 edges per core per relation
GCH = 4096                 # edges per gather chunk
CCH = 512                  # edges per compute chunk
NREL = 2
NPAIR = 1792               # u-row pairs (2 rows / 512B descriptor) per chunk
NOCT = 0                   # u-row octs: disabled (4-way gather split hurt pipelining)

_PROGRAM_CACHE = {}
LAST_RESULTS = None


def _build_program(nsub, nchunk, tabrows, subl, npair, noct):
    import concourse.bacc as bacc
    import concourse.bass as bass
    import concourse.mybir as mybir
    from concourse.tile import TileContext

    f16, f32, i16 = mybir.dt.float16, mybir.dt.float32, mybir.dt.int16
    subpad = nchunk * GCH
    # real (non-pad) index count per chunk; pads are -1 and the SWDGE ucode
    # stops descriptor generation at the last non-negative index
    counts = [min(GCH, subl - c * GCH) for c in range(nchunk)]
    # full chunks gather noct u-row OCTS (2KB descriptors, 8 adjacent rows),
    # npair PAIRS (512B, 2 rows) and singles; partial chunks are all singles
    nsing_full = GCH - 2 * npair - 8 * noct

    nc = bacc.Bacc("TRN2", target_bir_lowering=False, debug=False,
                   num_swdge_queues=4)

    tabs, idxs_d, outs = {}, {}, {}
    for r in range(NREL):
        for s in range(nsub):
            tabs[("u", r, s)] = nc.dram_tensor(
                f"ut{r}_{s}", [tabrows, D], f16, kind="ExternalInput")
            tabs[("v", r, s)] = nc.dram_tensor(
                f"vt{r}_{s}", [tabrows, D], f16, kind="ExternalInput")
            idxs_d[("u0", r, s)] = nc.dram_tensor(
                f"u0i{r}_{s}", [nchunk, P, max(noct // 16, 1)], i16,
                kind="ExternalInput")
            idxs_d[("u1", r, s)] = nc.dram_tensor(
                f"u1i{r}_{s}", [nchunk, P, max(npair // 16, 1)], i16,
                kind="ExternalInput")
            idxs_d[("u2", r, s)] = nc.dram_tensor(
                f"u2i{r}_{s}", [nchunk, P, GCH // 16], i16, kind="ExternalInput")
            idxs_d[("v", r, s)] = nc.dram_tensor(
                f"vi{r}_{s}", [nchunk, P, GCH // 16], i16, kind="ExternalInput")
        outs[r] = nc.dram_tensor(f"o{r}", [nsub, subpad], f32,
                                 kind="ExternalOutput")
    w1u_d = [nc.dram_tensor(f"w1u{r}", [D, HID], f16, kind="ExternalInput")
             for r in range(NREL)]
    w1v_d = [nc.dram_tensor(f"w1v{r}", [D, HID], f16, kind="ExternalInput")
             for r in range(NREL)]
    w2_d = [nc.dram_tensor(f"w2{r}", [P, 2], f16, kind="ExternalInput")
            for r in range(NREL)]
    b1_d = [nc.dram_tensor(f"b1{r}", [P, 2], f32, kind="ExternalInput")
            for r in range(NREL)]
    b2_d = [nc.dram_tensor(f"b2{r}", [1, 1], f32, kind="ExternalInput")
            for r in range(NREL)]
    id_d = nc.dram_tensor("ident", [P, P], f16, kind="ExternalInput")

    with TileContext(nc) as tc:
        with tc.tile_pool(name="sbw", bufs=1) as sbw, \
             tc.tile_pool(name="sbi", bufs=6) as sbi, \
             tc.tile_pool(name="sbg", bufs=4) as sbg, \
             tc.tile_pool(name="sbt", bufs=3) as sbt, \
             tc.tile_pool(name="sbh", bufs=4) as sbh, \
             tc.tile_pool(name="sblog", bufs=2) as sblog, \
             tc.tile_pool(name="pt", bufs=2, space="PSUM") as pt, \
             tc.tile_pool(name="ph", bufs=3, space="PSUM") as ph, \
             tc.tile_pool(name="pl", bufs=2, space="PSUM") as pl:

            w1u_t, w1v_t, w2_t, b1_t, b2_t = [], [], [], [], []
            for r in range(NREL):
                t = sbw.tile([D, HID], f16, tag=f"w1u{r}")
                nc.sync.dma_start(out=t[:], in_=w1u_d[r].ap()[:])
                w1u_t.append(t)
                t = sbw.tile([D, HID], f16, tag=f"w1v{r}")
                nc.sync.dma_start(out=t[:], in_=w1v_d[r].ap()[:])
                w1v_t.append(t)
                t = sbw.tile([P, 2], f16, tag=f"w2{r}")
                nc.sync.dma_start(out=t[:], in_=w2_d[r].ap()[:])
                w2_t.append(t)
                t = sbw.tile([P, 2], f32, tag=f"b1{r}")
                nc.sync.dma_start(out=t[:], in_=b1_d[r].ap()[:])
                b1_t.append(t)
                t = sbw.tile([1, 1], f32, tag=f"b2{r}")
                nc.sync.dma_start(out=t[:], in_=b2_d[r].ap()[:])
                b2_t.append(t)
            ident = sbw.tile([P, P], f16, tag="ident")
            nc.sync.dma_start(out=ident[:], in_=id_d.ap()[:])

            q = 0
            for r in range(NREL):
                for s in range(nsub):
                    utab = tabs[("u", r, s)]
                    # overlapping f32 view: row stride D fp16, 2 rows (512B =
                    # 128 f32 elems) per read — the SWDGE ucode costs ~8ns per
                    # 128-element unit, so an f32 view makes a 2-row fetch as
                    # cheap as a 1-row one
                    utab_pair = bass.AP(utab.ap().tensor, 0,
                                        [[D, tabrows - 1], [1, 2 * D]]
                                        ).bitcast(mybir.dt.float32)
                    utab_oct = bass.AP(utab.ap().tensor, 0,
                                       [[D, tabrows - 7], [1, 8 * D]]
                                       ).bitcast(mybir.dt.float32)
                    for c in range(nchunk):
                        full = counts[c] == GCH and npair > 0
                        gu = sbg.tile([P, GCH // P, D], f16, tag="gu")
                        ob = 8 * noct // P      # blocks used by octs
                        pb = 2 * npair // P     # blocks used by pairs
                        if full:
                            if noct:
                                u0_t = sbi.tile([P, noct // 16], i16, tag="u0")
                                nc.sync.dma_start(out=u0_t[:],
                                                  in_=idxs_d[("u0", r, s)].ap()[c])
                            u1_t = sbi.tile([P, npair // 16], i16, tag="u1")
                            nc.sync.dma_start(out=u1_t[:],
                                              in_=idxs_d[("u1", r, s)].ap()[c])
                            u2_t = sbi.tile([P, nsing_full // 16], i16, tag="u2")
                            nc.sync.dma_start(
                                out=u2_t[:],
                                in_=idxs_d[("u2", r, s)].ap()[c][:, :nsing_full // 16])
                            if noct:
                                oct_out = gu[:, 0:ob, :].rearrange(
                                    "p (a eight) d -> p a (eight d)", eight=8
                                ).bitcast(mybir.dt.float32)
                                nc.gpsimd.dma_gather(
                                    oct_out, utab_oct, u0_t[:],
                                    noct, noct, 4 * D, elem_step=D // 2,
                                    single_packet=False, queue_num=q % 4)
                            pair_out = gu[:, ob:ob + pb, :].rearrange(
                                "p (a two) d -> p a (two d)", two=2
                            ).bitcast(mybir.dt.float32)
                            no = 1 if noct else 0
                            nc.gpsimd.dma_gather(
                                pair_out, utab_pair, u1_t[:],
                                npair, npair, D, elem_step=D // 2,
                                single_packet=False, queue_num=(q + no) % 4)
                            nc.gpsimd.dma_gather(
                                gu[:, ob + pb:, :], utab.ap()[:], u2_t[:],
                                nsing_full, nsing_full, D,
                                single_packet=False, queue_num=(q + no + 1) % 4)
                            qv = (q + no + 2) % 4
                            q += no + 3
                        else:
                            u2_t = sbi.tile([P, GCH // 16], i16, tag="u2")
                            nc.sync.dma_start(out=u2_t[:],
                                              in_=idxs_d[("u2", r, s)].ap()[c])
                            nc.gpsimd.dma_gather(
                                gu[:], utab.ap()[:], u2_t[:],
                                GCH, counts[c], D, single_packet=False,
                                queue_num=q % 4)
                            qv = (q + 1) % 4
                            q += 2
                        vi_t = sbi.tile([P, GCH // 16], i16, tag="vi")
                        nc.sync.dma_start(out=vi_t[:],
                                          in_=idxs_d[("v", r, s)].ap()[c])
                        gv = sbg.tile([P, GCH // P, D], f16, tag="gv")
                        if full:
                            # split v into two half-gathers on separate queues
                            # so per-queue descriptor loads stay balanced
                            h = GCH // 2
                            nc.gpsimd.dma_gather(
                                gv[:, :GCH // P // 2, :],
                                tabs[("v", r, s)].ap()[:], vi_t[:, :h // 16],
                                h, h, D, single_packet=False, queue_num=qv)
                            nc.gpsimd.dma_gather(
                                gv[:, GCH // P // 2:, :],
                                tabs[("v", r, s)].ap()[:], vi_t[:, h // 16:],
                                h, h, D, single_packet=False,
                                queue_num=(qv + 1) % 4)
                            q += 1
                        else:
                            nc.gpsimd.dma_gather(
                                gv[:], tabs[("v", r, s)].ap()[:], vi_t[:],
                                GCH, counts[c], D, single_packet=False,
                                queue_num=qv)

                        log_sb = sblog.tile([1, GCH], f32, tag="log")
                        ncc = -(-counts[c] // CCH)
                        for cc in range(ncc):
                            ptu = pt.tile([P, CCH], f16, tag="pt")
                            for j in range(CCH // P):
                                nc.tensor.transpose(
                                    out=ptu[:, j * P:(j + 1) * P],
                                    in_=gu[:, cc * (CCH // P) + j, :],
                                    identity=ident[:])
                            tu = sbt.tile([P, CCH], f16, tag="tu")
                            nc.vector.tensor_copy(out=tu[:], in_=ptu[:])
                            ptv = pt.tile([P, CCH], f16, tag="pt")
                            for j in range(CCH // P):
                                nc.tensor.transpose(
                                    out=ptv[:, j * P:(j + 1) * P],
                                    in_=gv[:, cc * (CCH // P) + j, :],
                                    identity=ident[:])
                            tv = sbt.tile([P, CCH], f16, tag="tv")
                            nc.vector.tensor_copy(out=tv[:], in_=ptv[:])

                            hts = []
                            for hc in range(2):
                                php = ph.tile([P, CCH], f32, tag="ph")
                                nc.tensor.matmul(
                                    out=php[:],
                                    lhsT=w1u_t[r][:, hc * P:(hc + 1) * P],
                                    rhs=tu[:], start=True, stop=False)
                                nc.tensor.matmul(
                                    out=php[:],
                                    lhsT=w1v_t[r][:, hc * P:(hc + 1) * P],
                                    rhs=tv[:], start=False, stop=True)
                                ht = sbh.tile([P, CCH], f16, tag="ht")
                                nc.scalar.activation(
                                    out=ht[:], in_=php[:],
                                    func=mybir.ActivationFunctionType.Relu,
                                    bias=b1_t[r][:, hc:hc + 1])
                                hts.append(ht)
                            plt = pl.tile([1, CCH], f32, tag="pl")
                            nc.tensor.matmul(out=plt[:], lhsT=w2_t[r][:, 0:1],
                                             rhs=hts[0][:], start=True, stop=False)
                            nc.tensor.matmul(out=plt[:], lhsT=w2_t[r][:, 1:2],
                                             rhs=hts[1][:], start=False, stop=True)
                            nc.scalar.activation(
                                out=log_sb[:, cc * CCH:(cc + 1) * CCH],
                                in_=plt[:],
                                func=mybir.ActivationFunctionType.Identity,
                                bias=b2_t[r][:])
                        nc.sync.dma_start(
                            out=outs[r].ap()[s:s + 1,
                                             c * GCH:c * GCH + ncc * CCH],
                            in_=log_sb[:, :ncc * CCH])
    nc.compile()
    return nc


def _wrap16(idx16, nchunk):
    """[subpad] int16 -> [nchunk, 128, GCH//16]: stream pos g of chunk c sits
    at partition g%16 (replicated to all 8 Q7 core groups), column g//16."""
    a = idx16.reshape(nchunk, GCH // 16, 16)
    a = np.swapaxes(a, 1, 2)                       # [nchunk, 16, GCH//16]
    return np.tile(a, (1, 8, 1)).copy()            # [nchunk, 128, GCH//16]


def _wrap16_row(idx16):
    """[n] int16 -> [128, n//16] (16-wrap, replicated to 8 core groups)."""
    a = idx16.reshape(-1, 16).T
    return np.tile(a, (8, 1)).copy()


def _prep(user_embed, item_embed, u_clicks, v_clicks, u_buys, v_buys,
          W1_clicks, b1_clicks, W2_clicks, b2_clicks,
          W1_buys, b1_buys, W2_buys, b2_buys):
    user_embed = np.asarray(user_embed, dtype=np.float32)
    item_embed = np.asarray(item_embed, dtype=np.float32)
    rels = [
        (np.asarray(u_clicks), np.asarray(v_clicks),
         np.asarray(W1_clicks, np.float32), np.asarray(b1_clicks, np.float32),
         np.asarray(W2_clicks, np.float32), np.asarray(b2_clicks, np.float32)),
        (np.asarray(u_buys), np.asarray(v_buys),
         np.asarray(W1_buys, np.float32), np.asarray(b1_buys, np.float32),
         np.asarray(W2_buys, np.float32), np.asarray(b2_buys, np.float32)),
    ]
    user16 = user_embed.astype(np.float16)
    item16 = item_embed.astype(np.float16)

    # pick nsub so every sub-shard's unique index count fits int16
    nsub = 2
    while True:
        subl = EPC // nsub
        ok = True
        for r in range(NREL):
            u_all, v_all = rels[r][0], rels[r][1]
            for k in range(NCORES):
                for s in range(nsub):
                    lo = k * EPC + s * subl
                    hi = lo + subl
                    if len(np.unique(u_all[lo:hi])) > 32700 or \
                       len(np.unique(v_all[lo:hi])) > 32700:
                        ok = False
                        break
                if not ok:
                    break
            if not ok:
                break
        if ok:
            break
        nsub *= 2
        if nsub > 16:
            raise RuntimeError("index space too dense for int16 gather")
    subl = EPC // nsub
    nchunk = -(-subl // GCH)          # chunks per sub-shard
    subpad = nchunk * GCH
    tabrows = 32768

    counts = [min(GCH, subl - c * GCH) for c in range(nchunk)]
    npair = NPAIR
    noct = NOCT
    nsing_full = GCH - 2 * npair - 8 * noct

    def _greedy(d, L, cap, used):
        starts, j, N = [], 0, len(d) + 1
        while j + L <= N and len(starts) < cap:
            if not used[j:j + L].any() and (d[j:j + L - 1] == 1).all():
                starts.append(j)
                used[j:j + L] = True
                j += L
            else:
                j += 1
        return np.asarray(starts, np.int64)

    def _shard(u_sub, v_sub):
        perm = np.argsort(u_sub, kind="stable")
        us, vs = u_sub[perm], v_sub[perm]
        uniq_u, pos = np.unique(us, return_inverse=True)
        uniq_v, vinv = np.unique(vs, return_inverse=True)
        u0 = np.zeros((nchunk, max(noct, 1)), np.int16)
        u1 = np.zeros((nchunk, max(npair, 1)), np.int16)
        u2 = np.full((nchunk, GCH), -1, np.int16)
        v_dev = np.full(subpad, -1, np.int64)
        ood = np.full(subpad, -1, np.int64)
        ob = 8 * noct // P
        pb = 2 * npair // P
        for c in range(nchunk):
            base, cnt = c * GCH, counts[c]
            if cnt == GCH and npair > 0:
                pp = pos[base:base + GCH]
                d = np.diff(pp)
                used = np.zeros(GCH, bool)
                octs = _greedy(d, 8, noct, used)
                pairs = _greedy(d, 2, npair, used)
                if len(octs) < noct or len(pairs) < npair:
                    return None
                sing = np.where(~used)[0]
                if noct:
                    u0[c] = pos[base + octs].astype(np.int16)
                u1[c] = pos[base + pairs].astype(np.int16)
                u2[c, :nsing_full] = pos[base + sing].astype(np.int16)
                ks = np.arange(noct)
                js = np.arange(npair)
                ss = np.arange(nsing_full)
                place = []
                for i in range(8):
                    place.append(((8 * (ks // P) + i) * P + ks % P, octs + i))
                gA = (ob + 2 * (js // P)) * P + js % P
                place.append((gA, pairs))
                place.append((gA + P, pairs + 1))
                place.append(((ob + pb + ss // P) * P + ss % P, sing))
                for g, e in place:
                    e = base + e
                    v_dev[base + g] = vinv[e]
                    ood[base + g] = perm[e]
            else:
                e = base + np.arange(cnt)
                u2[c, :cnt] = pos[e].astype(np.int16)
                v_dev[base:base + cnt] = vinv[e]
                ood[base:base + cnt] = perm[e]
        return u0, u1, u2, v_dev, ood, uniq_u, uniq_v

    # verify pairing feasibility on all shards first
    shards = {}
    feasible = True
    for r in range(NREL):
        u_all, v_all = rels[r][0], rels[r][1]
        for k in range(NCORES):
            for s in range(nsub):
                lo = k * EPC + s * subl
                sh = _shard(np.asarray(u_all[lo:lo + subl], np.int64),
                            np.asarray(v_all[lo:lo + subl], np.int64))
                if sh is None:
                    feasible = False
                    break
                shards[(k, r, s)] = sh
            if not feasible:
                break
        if not feasible:
            break
    if not feasible:
        npair = 0
        noct = 0
        nsing_full = GCH
        shards = {}
        for r in range(NREL):
            u_all, v_all = rels[r][0], rels[r][1]
            for k in range(NCORES):
                for s in range(nsub):
                    lo = k * EPC + s * subl
                    shards[(k, r, s)] = _shard(
                        np.asarray(u_all[lo:lo + subl], np.int64),
                        np.asarray(v_all[lo:lo + subl], np.int64))

    in_maps, scat = [], {}
    for k in range(NCORES):
        m = {"ident": np.eye(P, dtype=np.float16)}
        for r in range(NREL):
            u_all, v_all, W1, b1, W2, b2 = rels[r]
            m[f"w1u{r}"] = W1[:D].astype(np.float16)
            m[f"w1v{r}"] = W1[D:].astype(np.float16)
            m[f"w2{r}"] = W2.reshape(2, P).T.astype(np.float16).copy()
            m[f"b1{r}"] = b1.reshape(2, P).T.astype(np.float32).copy()
            m[f"b2{r}"] = b2.reshape(1, 1).astype(np.float32)
            for s in range(nsub):
                u0, u1, u2, v_dev, ood, uniq_u, uniq_v = shards[(k, r, s)]
                comp = np.zeros((tabrows, D), np.float16)
                comp[:len(uniq_u)] = user16[uniq_u]
                m[f"ut{r}_{s}"] = comp
                comp = np.zeros((tabrows, D), np.float16)
                comp[:len(uniq_v)] = item16[uniq_v]
                m[f"vt{r}_{s}"] = comp
                if npair > 0:
                    if noct > 0:
                        m[f"u0i{r}_{s}"] = np.stack(
                            [_wrap16_row(u0[c]) for c in range(nchunk)])
                    else:
                        m[f"u0i{r}_{s}"] = np.zeros((nchunk, P, 1), np.int16)
                    m[f"u1i{r}_{s}"] = np.stack(
                        [_wrap16_row(u1[c]) for c in range(nchunk)])
                else:
                    m[f"u0i{r}_{s}"] = np.zeros((nchunk, P, 1), np.int16)
                    m[f"u1i{r}_{s}"] = np.zeros((nchunk, P, 1), np.int16)
                m[f"u2i{r}_{s}"] = np.stack(
                    [_wrap16_row(u2[c]) for c in range(nchunk)])
                m[f"vi{r}_{s}"] = _wrap16(
                    np.where(v_dev >= 0, v_dev, -1).astype(np.int16), nchunk)
                scat[(k, r, s)] = ood
        in_maps.append(m)
    return nsub, nchunk, subl, subpad, tabrows, npair, noct, in_maps, scat


def make_in_maps(np_inputs):
    """For external harnesses: per-core input maps for the cached program."""
    return _prep(**np_inputs)[7]


def kernel(**inputs):
    global LAST_RESULTS
    from concourse import bass_utils

    nsub, nchunk, subl, subpad, tabrows, npair, noct, in_maps, scat = _prep(**inputs)

    key = (nsub, nchunk, tabrows, subl, npair, noct)
    if key not in _PROGRAM_CACHE:
        _PROGRAM_CACHE[key] = _build_program(nsub, nchunk, tabrows, subl, npair, noct)
    nc = _PROGRAM_CACHE[key]

    res = bass_utils.run_bass_kernel_spmd(nc, in_maps, core_ids=list(range(NCORES)))
    LAST_RESULTS = res

    outs = []
    for r in range(NREL):
        full = np.empty(E, np.float32)
        for k in range(NCORES):
            o = res.results[k][f"o{r}"]          # [nsub, subpad]
            for s in range(nsub):
                lo = k * EPC + s * subl
                ood = scat[(k, r, s)]
                valid = ood >= 0
                full[lo + ood[valid]] = o[s][valid]
        outs.append(full)
    return outs[0], outs[1]
